# revision 12
# baseline (speedup 1.0000x reference)
"""Trainium2 Bass kernel for a multi-head self-attention block.

Reference computation (B=4, N=2048, D=256, H=8, dh=32, DFF=512):
    x_ln = LN0(x); Q = x_ln@Wq.T+bq; K = y@Wk.T+bk; V = y@Wv.T+bv
    per head: A = softmax(Qh Kh^T / 16); O = concat_h(Qh + A Vh)
    out = O + (gelu(LN1(O)@W1.T+b1) @ W2.T + b2)

Sharding: 8 cores = 4 batches x 2 halves of the query sequence. Each core
gets its x half-shard and the full y for its batch; no collectives.

Layout: feature-on-partition ("transposed") everywhere. The 256 feature
dims of Q/O are spread over a 512-slot space [128 partitions, 4 ktiles]:
head h lives at partition strip 64*(h%2)..+32, ktile o=h//2 (the other
strips are zero). This puts every head's attention output exactly where
the PE col-packed AV matmul (M=33, tile_position col in {0,64}) can
write it, with the softmax denominator coming for free from a ones
column appended to V (row 32/96 of the AV accumulator). LN folds, head
permutation, and the V-bias fold (bv moves into bq since sum(A)=1) are
all host-side weight prep. No max-subtraction in softmax (|s/16|<~1.5).
"""

import contextlib

import numpy as np

B, N, D = 4, 2048, 256
H, DH, DFF = 8, 32, 512
P = 128
NTOK = N // 2            # query tokens per core
NQT = NTOK // 512        # q tiles of 512
NKT = N // P             # key tiles of 128
SCALE = 1.0 / 16.0
EPS = 1e-5
DSLOT = 512              # padded feature-slot space for Q/K/O

_NC_CACHE = {}


def _slot(h, i):
    return (h // 2) * P + 64 * (h % 2) + i


def _build_nc():
    import concourse.mybir as mybir
    import concourse.tile as tile
    from concourse import bacc

    f32 = mybir.dt.float32
    f32r = mybir.dt.float32r
    AF = mybir.ActivationFunctionType
    ALU = mybir.AluOpType

    def R(ap):
        return ap.bitcast(f32r)

    nc = bacc.Bacc("TRN2", target_bir_lowering=False, debug=False)

    xt_d = nc.dram_tensor("xt", [D, NTOK], f32, kind="ExternalInput")
    yt_d = nc.dram_tensor("yt", [D, N], f32, kind="ExternalInput")
    wq_d = nc.dram_tensor("wq", [D, DSLOT], f32, kind="ExternalInput")
    bq_d = nc.dram_tensor("bq", [DSLOT], f32, kind="ExternalInput")
    wk_d = nc.dram_tensor("wk", [D, DSLOT], f32, kind="ExternalInput")
    bk_d = nc.dram_tensor("bk", [DSLOT], f32, kind="ExternalInput")
    wv_d = nc.dram_tensor("wv", [D, H * 33], f32, kind="ExternalInput")
    w1_d = nc.dram_tensor("w1", [DSLOT, DFF], f32, kind="ExternalInput")
    b1_d = nc.dram_tensor("b1", [DFF], f32, kind="ExternalInput")
    w2_d = nc.dram_tensor("w2", [DFF + 1, DSLOT], f32, kind="ExternalInput")
    out_d = nc.dram_tensor("out_t", [D, NTOK], f32, kind="ExternalOutput")

    with tile.TileContext(nc) as tc, contextlib.ExitStack() as ctx:
        const = ctx.enter_context(tc.tile_pool(name="const", bufs=1))
        big = ctx.enter_context(tc.tile_pool(name="big", bufs=1))
        scratch = ctx.enter_context(tc.tile_pool(name="scratch", bufs=1))
        apool = ctx.enter_context(tc.tile_pool(name="apool", bufs=3))
        # PSUM: scores 2x[128,1024]=4 banks, av 2, bc 1, proj 1.
        scores_pool = ctx.enter_context(
            tc.tile_pool(name="scoresp", bufs=2, space="PSUM"))
        av_pool = ctx.enter_context(tc.tile_pool(name="avp", bufs=2, space="PSUM"))
        bc_pool = ctx.enter_context(tc.tile_pool(name="bcp", bufs=1, space="PSUM"))
        proj_pool = ctx.enter_context(tc.tile_pool(name="projp", bufs=1, space="PSUM"))

        # ---- constants / inputs -------------------------------------------
        ones_s = const.tile([P, 512], f32)
        nc.vector.memset(ones_s[:], 1.0)
        eps_s = const.tile([1, 1], f32)
        nc.vector.memset(eps_s[:], EPS)

        xt_s = big.tile([P, 2, NTOK], f32)
        nc.sync.dma_start(xt_s[:], xt_d.rearrange("(o p) t -> p o t", p=P))
        yt_s = big.tile([P, 2, N], f32)
        nc.sync.dma_start(yt_s[:], yt_d.rearrange("(o p) t -> p o t", p=P))

        wq_s = const.tile([P, 2, DSLOT], f32)
        nc.sync.dma_start(wq_s[:], wq_d.rearrange("(o p) m -> p o m", p=P))
        wk_s = const.tile([P, 2, DSLOT], f32)
        nc.sync.dma_start(wk_s[:], wk_d.rearrange("(o p) m -> p o m", p=P))
        wv_s = const.tile([P, 2, H * 33], f32)
        nc.sync.dma_start(wv_s[:], wv_d.rearrange("(o p) m -> p o m", p=P))
        w1_s = const.tile([P, 4, DFF], f32)
        nc.sync.dma_start(w1_s[:], w1_d.rearrange("(o p) m -> p o m", p=P))
        w2_s = const.tile([P, 5, DSLOT], f32)
        nc.sync.dma_start(w2_s[:, 0:4, :],
                          w2_d[0:DFF, :].rearrange("(o p) m -> p o m", p=P))
        nc.sync.dma_start(w2_s[0:1, 4, :], w2_d[DFF:, :])
        bq_s = const.tile([P, 4], f32)
        nc.sync.dma_start(bq_s[:], bq_d.rearrange("(m p) -> p m", p=P))
        bk_s = const.tile([P, 4], f32)
        nc.sync.dma_start(bk_s[:], bk_d.rearrange("(m p) -> p m", p=P))
        b1_s = const.tile([P, 4], f32)
        nc.sync.dma_start(b1_s[:], b1_d.rearrange("(m p) -> p m", p=P))

        # ---- helper: layernorm over the partition-tiled feature dim --------
        def layernorm(src, dst, no, sq):
            """src/dst/sq: [128, no, NTOK]; normalize over the feature rows
            of each token column (zero rows contribute 0 to the sums; divide
            by the true D=256). sq is borrowed scratch storage."""
            nc.scalar.activation(out=sq[:], in_=src[:], func=AF.Square)
            mean = scratch.tile([1, NTOK], f32, tag="mean")
            rstd = scratch.tile([1, NTOK], f32, tag="rstd")
            tmp = scratch.tile([1, NTOK], f32, tag="lntmp")
            for hf in range(NTOK // 512):
                cs = slice(hf * 512, hf * 512 + 512)
                sx_ps = av_pool.tile([1, 512], f32, tag="av")
                sq_ps = bc_pool.tile([1, 512], f32, tag="bc")
                for o in range(no):
                    nc.tensor.matmul(sx_ps[:], lhsT=R(ones_s[:, 0:1]),
                                     rhs=R(src[:, o, cs]),
                                     start=(o == 0), stop=(o == no - 1))
                    nc.tensor.matmul(sq_ps[:], lhsT=R(ones_s[:, 0:1]),
                                     rhs=R(sq[:, o, cs]),
                                     start=(o == 0), stop=(o == no - 1))
                nc.vector.tensor_scalar_mul(mean[0:1, cs], sx_ps[:], 1.0 / D)
                nc.vector.tensor_scalar_mul(tmp[0:1, cs], sq_ps[:], 1.0 / D)
            m2 = scratch.tile([1, NTOK], f32, tag="m2")
            nc.vector.tensor_tensor(out=m2[:], in0=mean[:], in1=mean[:],
                                    op=ALU.mult)
            nc.vector.tensor_tensor(out=tmp[:], in0=tmp[:], in1=m2[:],
                                    op=ALU.subtract)
            nc.scalar.activation(out=tmp[:], in_=tmp[:], func=AF.Sqrt,
                                 bias=eps_s[:])
            nc.vector.reciprocal(out=rstd[:], in_=tmp[:])
            meanb = scores_pool.tile([P, 1024], f32, tag="scores", name="mb")
            rstdb = scores_pool.tile([P, 1024], f32, tag="scores", name="rb")
            for hf in range(NTOK // 512):
                cs = slice(hf * 512, hf * 512 + 512)
                nc.tensor.matmul(meanb[:, cs], lhsT=R(ones_s[0:1, 0:P]),
                                 rhs=R(mean[0:1, cs]), start=True, stop=True)
                nc.tensor.matmul(rstdb[:, cs], lhsT=R(ones_s[0:1, 0:P]),
                                 rhs=R(rstd[0:1, cs]), start=True, stop=True)
            for o in range(no):
                nc.vector.tensor_tensor(out=dst[:, o, :], in0=src[:, o, :],
                                        in1=meanb[:], op=ALU.subtract)
                nc.vector.tensor_tensor(out=dst[:, o, :], in0=dst[:, o, :],
                                        in1=rstdb[:], op=ALU.mult)

        # ---- phase A: LN0, Q/K/V projections -------------------------------
        xln_s = big.tile([P, 2, NTOK], f32)
        oln_s = big.tile([P, 4, NTOK], f32)
        layernorm(xt_s, xln_s, 2, oln_s[:, 0:2, :])   # oln as scratch for now

        qt_s = big.tile([P, 4, NTOK], f32)
        for mt in range(4):
            for nt in range(NQT):
                ns_ = slice(nt * 512, nt * 512 + 512)
                ps = proj_pool.tile([P, 512], f32, tag="proj", name="ps")
                for o in range(2):
                    nc.tensor.matmul(ps[:], lhsT=R(wq_s[:, o, mt * P:mt * P + P]),
                                     rhs=R(xln_s[:, o, ns_]),
                                     start=(o == 0), stop=(o == 1))
                nc.vector.tensor_scalar_add(qt_s[:, mt, ns_], ps[:],
                                            bq_s[:, mt:mt + 1])
        kt_s = big.tile([P, 4, N], f32)
        for mt in range(4):
            for nt in range(N // 512):
                ns_ = slice(nt * 512, nt * 512 + 512)
                ps = proj_pool.tile([P, 512], f32, tag="proj", name="ps")
                for o in range(2):
                    nc.tensor.matmul(ps[:], lhsT=R(wk_s[:, o, mt * P:mt * P + P]),
                                     rhs=R(yt_s[:, o, ns_]),
                                     start=(o == 0), stop=(o == 1))
                nc.vector.tensor_scalar_add(kt_s[:, mt, ns_], ps[:],
                                            bk_s[:, mt:mt + 1])
        # V in natural [token, dout] layout, 33-wide head blocks ([Vh | ones])
        v_s = big.tile([P, NKT, H * 33], f32)
        for tt in range(NKT):
            ts_ = slice(tt * P, tt * P + P)
            ps = proj_pool.tile([P, 512], f32, tag="proj", name="ps")[:, 0:H * 33]
            for o in range(2):
                nc.tensor.matmul(ps[:], lhsT=R(yt_s[:, o, ts_]),
                                 rhs=R(wv_s[:, o, :]), start=(o == 0), stop=(o == 1))
            nc.vector.tensor_copy(out=v_s[:, tt, :], in_=ps[:])
        for h in range(H):
            nc.vector.memset(v_s[:, :, 33 * h + 32], 1.0)

        # ---- phase B: attention -------------------------------------------
        ot_s = big.tile([P, 4, NTOK], f32)
        # zero the unwritten strips once (rows 32:64 and 96:128 of each o)
        nc.gpsimd.memset(ot_s[32:64, :, :], 0.0)
        nc.gpsimd.memset(ot_s[96:128, :, :], 0.0)
        rc_s = scratch.tile([P, 512], f32, tag="rc")
        for pr in range(4):              # head pair: heads {2pr, 2pr+1}
            for qt in range(NQT):
                qs_ = slice(qt * 512, qt * 512 + 512)
                av = av_pool.tile([P, 512], f32, tag="av", name="av")
                for kt in range(NKT):
                    ks_ = slice(kt * P, kt * P + P)
                    sp = scores_pool.tile([P, 1024], f32, tag="scores",
                                          name="sp")
                    for jj in range(2):
                        st = 64 * jj
                        nc.tensor.matmul(
                            sp[:, jj * 512:jj * 512 + 512],
                            lhsT=R(kt_s[st:st + 32, pr, ks_]),
                            rhs=R(qt_s[st:st + 32, pr, qs_]),
                            start=True, stop=True,
                            tile_position=(st, 0))
                    a = apool.tile([P, 1024], f32, tag="a", name="a")
                    nc.scalar.activation(out=a[:], in_=sp[:], func=AF.Exp,
                                         scale=SCALE)
                    for jj in range(2):
                        h = 2 * pr + jj
                        st = 64 * jj
                        nc.tensor.matmul(
                            av[st:st + 33, :],
                            lhsT=R(v_s[:, kt, 33 * h:33 * h + 33]),
                            rhs=R(a[:, jj * 512:jj * 512 + 512]),
                            start=(kt == 0), stop=(kt == NKT - 1),
                            tile_position=(0, st),
                            skip_group_check=True)
                # normalize by the ones-column sums + per-head residual with Q
                bc = bc_pool.tile([P, 512], f32, tag="bc", name="bc")
                for jj in range(2):
                    st = 64 * jj
                    nc.vector.reciprocal(out=rc_s[st + 32:st + 33, :],
                                         in_=av[st + 32:st + 33, :])
                    nc.tensor.matmul(bc[st:st + 32, :],
                                     lhsT=R(ones_s[st + 32:st + 33, 0:32]),
                                     rhs=R(rc_s[st + 32:st + 33, :]),
                                     start=True, stop=True,
                                     tile_position=(st + 32, st))
                avs = scratch.tile([P, 512], f32, tag="avs", name="avs")
                nrm = scratch.tile([P, 512], f32, tag="nrm", name="nrm")
                for jj in range(2):
                    st = 64 * jj
                    nc.vector.tensor_copy(out=avs[st:st + 32, :],
                                          in_=av[st:st + 32, :])
                    nc.vector.tensor_tensor(out=nrm[st:st + 32, :],
                                            in0=avs[st:st + 32, :],
                                            in1=bc[st:st + 32, :],
                                            op=ALU.mult)
                    nc.vector.tensor_tensor(out=ot_s[st:st + 32, pr, qs_],
                                            in0=nrm[st:st + 32, :],
                                            in1=qt_s[st:st + 32, pr, qs_],
                                            op=ALU.add)

        # ---- phase C: LN1 + FFN + final residual ---------------------------
        # reuse yt_s storage (dead after K/V proj) for the FFN hidden acts
        h_s = yt_s[:].rearrange("p o t -> p (o t)").rearrange(
            "p (o t) -> p o t", o=4)
        layernorm(ot_s, oln_s, 4, h_s)
        for mt in range(DFF // P):
            ms = slice(mt * P, mt * P + P)
            for nt in range(NQT):
                ns_ = slice(nt * 512, nt * 512 + 512)
                ps = proj_pool.tile([P, 512], f32, tag="proj", name="ps")
                for o in range(4):
                    nc.tensor.matmul(ps[:], lhsT=R(w1_s[:, o, ms]),
                                     rhs=R(oln_s[:, o, ns_]),
                                     start=(o == 0), stop=(o == 3))
                nc.scalar.activation(out=h_s[:, mt, ns_], in_=ps[:],
                                     func=AF.Gelu, bias=b1_s[:, mt:mt + 1])

        # reuse qt_s storage (dead after attention) for the final output
        outt_s = qt_s
        for mt in range(4):
            ms = slice(mt * P, mt * P + P)
            for nt in range(NQT):
                ns_ = slice(nt * 512, nt * 512 + 512)
                ps = proj_pool.tile([P, 512], f32, tag="proj", name="ps")
                for o in range(4):
                    nc.tensor.matmul(ps[:], lhsT=R(w2_s[:, o, ms]),
                                     rhs=R(h_s[:, o, ns_]),
                                     start=(o == 0), stop=False)
                nc.tensor.matmul(ps[:], lhsT=R(w2_s[0:1, 4, ms]),
                                 rhs=R(ones_s[0:1, 0:512]), start=False, stop=True)
                nc.vector.tensor_tensor(out=outt_s[:, mt, ns_], in0=ps[:],
                                        in1=ot_s[:, mt, ns_], op=ALU.add)
        for h in range(H):
            nc.sync.dma_start(
                out_d[32 * h:32 * h + 32, :],
                outt_s[64 * (h % 2):64 * (h % 2) + 32, h // 2, :])

    nc.compile()
    return nc


def get_nc():
    if "nc" not in _NC_CACHE:
        _NC_CACHE["nc"] = _build_nc()
    return _NC_CACHE["nc"]


def _host_prep(inputs):
    f = lambda k: np.asarray(inputs[k], np.float32)
    x, y = f("x"), f("y")
    Wq, bq, Wk, bk, Wv, bv = f("Wq"), f("bq"), f("Wk"), f("bk"), f("Wv"), f("bv")
    W1, b1, W2, b2 = f("W1"), f("b1"), f("W2"), f("b2")
    ln0_g, ln0_b, ln1_g, ln1_b = f("ln0_g"), f("ln0_b"), f("ln1_g"), f("ln1_b")
    # fold LN affines into the following linears; fold bv into bq (sum(A)=1)
    Wq_eff = Wq * ln0_g[None, :]
    bq_eff = bq + Wq @ ln0_b + bv
    W1_eff = W1 * ln1_g[None, :]
    b1_eff = b1 + W1 @ ln1_b

    # permutation: original feature d=32h+i -> slot(h,i) in the 512 space
    slots = np.zeros(D, np.int64)
    for h in range(H):
        for i in range(DH):
            slots[DH * h + i] = _slot(h, i)

    wq_h = np.zeros((D, DSLOT), np.float32)
    wq_h[:, slots] = Wq_eff.T            # [din, dout-slot]
    bq_h = np.zeros(DSLOT, np.float32)
    bq_h[slots] = bq_eff
    wk_h = np.zeros((D, DSLOT), np.float32)
    wk_h[:, slots] = Wk.T
    bk_h = np.zeros(DSLOT, np.float32)
    bk_h[slots] = bk
    wv_h = np.zeros((D, H * 33), np.float32)
    for h in range(H):
        wv_h[:, 33 * h:33 * h + 32] = Wv.T[:, DH * h:DH * h + DH]
    w1_h = np.zeros((DSLOT, DFF), np.float32)
    w1_h[slots, :] = W1_eff.T            # [din-slot, dff]
    w2_h = np.zeros((DFF + 1, DSLOT), np.float32)
    w2_h[0:DFF, slots] = W2.T
    w2_h[DFF, slots] = b2

    in_maps = []
    for core in range(8):
        b, half = core // 2, core % 2
        in_maps.append({
            "xt": np.ascontiguousarray(x[b, half * NTOK:(half + 1) * NTOK, :].T),
            "yt": np.ascontiguousarray(y[b].T),
            "wq": wq_h, "bq": bq_h, "wk": wk_h, "bk": bk_h, "wv": wv_h,
            "w1": w1_h, "b1": np.ascontiguousarray(b1_eff), "w2": w2_h,
        })
    return in_maps


def kernel_with_results(inputs, **run_kwargs):
    from concourse.bass_utils import run_bass_kernel_spmd
    nc = get_nc()
    in_maps = _host_prep(inputs)
    res = run_bass_kernel_spmd(nc, in_maps, core_ids=list(range(8)), **run_kwargs)
    out = np.empty((B, N, D), np.float32)
    for core in range(8):
        b, half = core // 2, core % 2
        out[b, half * NTOK:(half + 1) * NTOK, :] = res.results[core]["out_t"].T
    return out, res


def kernel(**inputs):
    out, _ = kernel_with_results(inputs)
    return out



# revision 13
# speedup vs baseline: 1.9461x; 1.9461x over previous
"""Trainium2 Bass kernel for a multi-head self-attention block.

Reference computation (B=4, N=2048, D=256, H=8, dh=32, DFF=512):
    x_ln = LN0(x); Q = x_ln@Wq.T+bq; K = y@Wk.T+bk; V = y@Wv.T+bv
    per head: A = softmax(Qh Kh^T / 16); O = concat_h(Qh + A Vh)
    out = O + (gelu(LN1(O)@W1.T+b1) @ W2.T + b2)

Sharding: 8 cores = 4 batches x 2 halves of the query sequence. Each core
gets its x half-shard and the full y for its batch; no collectives.

Layout: feature-on-partition ("transposed") everywhere. The 256 feature
dims of Q/O are spread over a 512-slot space [128 partitions, 4 ktiles]:
head h lives at partition strip 64*(h%2)..+32, ktile o=h//2 (the other
strips are zero). This puts every head's attention output exactly where
the PE col-packed AV matmul (M=33, tile_position col in {0,64}) can
write it, with the softmax denominator coming for free from a ones
column appended to V (row 32/96 of the AV accumulator). LN folds, head
permutation, and the V-bias fold (bv moves into bq since sum(A)=1) are
all host-side weight prep. No max-subtraction in softmax (|s/16|<~1.5).

All matmul operands are bf16 (1 PE cycle/row vs 4 for fp32); PSUM
accumulation stays fp32. The softmax exp is split across engines: the
Act engine does exact Exp on most key-tiles, the DVE computes the rest
with a Schraudolph-style exp (one tensor_scalar building bf16 bit
patterns in int16; the +-3% equi-ripple error is common-mode across
the softmax and mostly cancels). GPSIMD takes the all-SBUF elementwise
work (LN normalize, residual adds) since it cannot touch PSUM.
"""

import contextlib

import numpy as np

B, N, D = 4, 2048, 256
H, DH, DFF = 8, 32, 512
P = 128
NTOK = N // 2            # query tokens per core
NQT = NTOK // 512        # q tiles of 512
NKT = N // P             # key tiles of 128
SCALE = 1.0 / 16.0
EPS = 1e-5
DSLOT = 512              # padded feature-slot space for Q/K/O

# Schraudolph exp constants for bf16 bit patterns in int16:
#   bits = round(s * SCALE*128*log2(e) + (127*128 - c8))
LOG2E = 1.4426950408889634
SCH_A = SCALE * 128.0 * LOG2E
SCH_B = 127.0 * 128.0 - 366392.5 / 65536.0
# key tiles handled by the DVE (Schraudolph) instead of Act (exact exp)
DVE_KT = frozenset((2, 5, 8, 11, 13, 15))

_NC_CACHE = {}


def _slot(h, i):
    return (h // 2) * P + 64 * (h % 2) + i


def _build_nc():
    import concourse.mybir as mybir
    import concourse.tile as tile
    from concourse import bacc

    f32 = mybir.dt.float32
    bf16 = mybir.dt.bfloat16
    i16 = mybir.dt.int16
    AF = mybir.ActivationFunctionType
    ALU = mybir.AluOpType

    nc = bacc.Bacc("TRN2", target_bir_lowering=False, debug=False)

    xt_d = nc.dram_tensor("xt", [D, NTOK], bf16, kind="ExternalInput")
    yt_d = nc.dram_tensor("yt", [D, N], bf16, kind="ExternalInput")
    wq_d = nc.dram_tensor("wq", [D, DSLOT], bf16, kind="ExternalInput")
    bq_d = nc.dram_tensor("bq", [DSLOT], f32, kind="ExternalInput")
    wk_d = nc.dram_tensor("wk", [D, DSLOT], bf16, kind="ExternalInput")
    bk_d = nc.dram_tensor("bk", [DSLOT], f32, kind="ExternalInput")
    wv_d = nc.dram_tensor("wv", [D, H * 33], bf16, kind="ExternalInput")
    w1_d = nc.dram_tensor("w1", [DSLOT, DFF], bf16, kind="ExternalInput")
    b1_d = nc.dram_tensor("b1", [DFF], f32, kind="ExternalInput")
    w2_d = nc.dram_tensor("w2", [DFF + 1, DSLOT], bf16, kind="ExternalInput")
    out_d = nc.dram_tensor("out_t", [D, NTOK], f32, kind="ExternalOutput")

    with tile.TileContext(nc) as tc, contextlib.ExitStack() as ctx:
        const = ctx.enter_context(tc.tile_pool(name="const", bufs=1))
        big = ctx.enter_context(tc.tile_pool(name="big", bufs=1))
        scratch = ctx.enter_context(tc.tile_pool(name="scratch", bufs=1))
        apool = ctx.enter_context(tc.tile_pool(name="apool", bufs=3))
        # PSUM: scores 2x[128,1024]=4 banks, av 2, bc 1, proj 1.
        scores_pool = ctx.enter_context(
            tc.tile_pool(name="scoresp", bufs=2, space="PSUM"))
        av_pool = ctx.enter_context(tc.tile_pool(name="avp", bufs=2, space="PSUM"))
        bc_pool = ctx.enter_context(tc.tile_pool(name="bcp", bufs=1, space="PSUM"))
        proj_pool = ctx.enter_context(tc.tile_pool(name="projp", bufs=1, space="PSUM"))

        # ---- constants / inputs -------------------------------------------
        ones_s = const.tile([P, 512], bf16)
        nc.vector.memset(ones_s[:], 1.0)
        eps_s = const.tile([1, 1], f32)
        nc.vector.memset(eps_s[:], EPS)

        xt_s = big.tile([P, 2, NTOK], bf16)
        nc.sync.dma_start(xt_s[:], xt_d.rearrange("(o p) t -> p o t", p=P))
        yt_s = big.tile([P, 2, N], bf16)
        nc.sync.dma_start(yt_s[:], yt_d.rearrange("(o p) t -> p o t", p=P))

        wq_s = const.tile([P, 2, DSLOT], bf16)
        nc.sync.dma_start(wq_s[:], wq_d.rearrange("(o p) m -> p o m", p=P))
        wk_s = const.tile([P, 2, DSLOT], bf16)
        nc.sync.dma_start(wk_s[:], wk_d.rearrange("(o p) m -> p o m", p=P))
        wv_s = const.tile([P, 2, H * 33], bf16)
        nc.sync.dma_start(wv_s[:], wv_d.rearrange("(o p) m -> p o m", p=P))
        w1_s = const.tile([P, 4, DFF], bf16)
        nc.sync.dma_start(w1_s[:], w1_d.rearrange("(o p) m -> p o m", p=P))
        w2_s = const.tile([P, 5, DSLOT], bf16)
        nc.sync.dma_start(w2_s[:, 0:4, :],
                          w2_d[0:DFF, :].rearrange("(o p) m -> p o m", p=P))
        nc.sync.dma_start(w2_s[0:1, 4, :], w2_d[DFF:, :])
        bq_s = const.tile([P, 4], f32)
        nc.sync.dma_start(bq_s[:], bq_d.rearrange("(m p) -> p m", p=P))
        bk_s = const.tile([P, 4], f32)
        nc.sync.dma_start(bk_s[:], bk_d.rearrange("(m p) -> p m", p=P))
        b1_s = const.tile([P, 4], f32)
        nc.sync.dma_start(b1_s[:], b1_d.rearrange("(m p) -> p m", p=P))

        # ---- helper: layernorm over the partition-tiled feature dim --------
        def layernorm(src, dst, no, sq):
            """src/dst/sq: [128, no, NTOK] bf16; normalize over the feature
            rows of each token column (zero rows contribute 0 to the sums;
            divide by the true D=256). sq is borrowed scratch storage."""
            nc.scalar.activation(out=sq[:], in_=src[:], func=AF.Square)
            mean = scratch.tile([1, NTOK], f32, tag="mean")
            mean_b = scratch.tile([1, NTOK], bf16, tag="mean_b")
            rstd_b = scratch.tile([1, NTOK], bf16, tag="rstd_b")
            tmp = scratch.tile([1, NTOK], f32, tag="lntmp")
            m2 = scratch.tile([1, NTOK], f32, tag="m2")
            for hf in range(NTOK // 512):
                cs = slice(hf * 512, hf * 512 + 512)
                sx_ps = av_pool.tile([1, 512], f32, tag="av")
                sq_ps = bc_pool.tile([1, 512], f32, tag="bc")
                for o in range(no):
                    nc.tensor.matmul(sx_ps[:], lhsT=ones_s[:, 0:1],
                                     rhs=src[:, o, cs],
                                     start=(o == 0), stop=(o == no - 1))
                    nc.tensor.matmul(sq_ps[:], lhsT=ones_s[:, 0:1],
                                     rhs=sq[:, o, cs],
                                     start=(o == 0), stop=(o == no - 1))
                nc.vector.tensor_scalar_mul(mean[0:1, cs], sx_ps[:], 1.0 / D)
                nc.vector.tensor_scalar_mul(tmp[0:1, cs], sq_ps[:], 1.0 / D)
            nc.vector.tensor_tensor(out=m2[:], in0=mean[:], in1=mean[:],
                                    op=ALU.mult)
            nc.vector.tensor_tensor(out=tmp[:], in0=tmp[:], in1=m2[:],
                                    op=ALU.subtract)
            nc.scalar.activation(out=tmp[:], in_=tmp[:], func=AF.Sqrt,
                                 bias=eps_s[:])
            with nc.allow_low_precision(reason="LN rstd in bf16"):
                nc.vector.reciprocal(out=rstd_b[:], in_=tmp[:])
            nc.vector.tensor_copy(out=mean_b[:], in_=mean[:])
            meanb = scores_pool.tile([P, 1024], f32, tag="scores", name="mb")
            rstdb = scores_pool.tile([P, 1024], f32, tag="scores", name="rb")
            for hf in range(NTOK // 512):
                cs = slice(hf * 512, hf * 512 + 512)
                nc.tensor.matmul(meanb[:, cs], lhsT=ones_s[0:1, 0:P],
                                 rhs=mean_b[0:1, cs], start=True, stop=True)
                nc.tensor.matmul(rstdb[:, cs], lhsT=ones_s[0:1, 0:P],
                                 rhs=rstd_b[0:1, cs], start=True, stop=True)
            mb_sb = scratch.tile([P, NTOK], bf16, tag="mb_sb")
            rb_sb = scratch.tile([P, NTOK], bf16, tag="rb_sb")
            nc.scalar.activation(out=mb_sb[:], in_=meanb[:], func=AF.Copy)
            nc.scalar.activation(out=rb_sb[:], in_=rstdb[:], func=AF.Copy)
            for o in range(no):
                nc.gpsimd.tensor_tensor(out=dst[:, o, :], in0=src[:, o, :],
                                        in1=mb_sb[:], op=ALU.subtract)
                nc.gpsimd.tensor_tensor(out=dst[:, o, :], in0=dst[:, o, :],
                                        in1=rb_sb[:], op=ALU.mult)

        # ---- phase A: LN0, K/V/Q projections -------------------------------
        xln_s = big.tile([P, 2, NTOK], bf16)
        oln_s = big.tile([P, 4, NTOK], bf16)
        layernorm(xt_s, xln_s, 2, oln_s[:, 0:2, :])   # oln as scratch for now

        # K/V first: they only need yt, so the PE keeps busy while the LN0
        # scalar chain finishes.
        kt_s = big.tile([P, 4, N], bf16)
        for mt in range(4):
            for nt in range(N // 512):
                ns_ = slice(nt * 512, nt * 512 + 512)
                ps = proj_pool.tile([P, 512], f32, tag="proj", name="ps")
                for o in range(2):
                    nc.tensor.matmul(ps[:], lhsT=wk_s[:, o, mt * P:mt * P + P],
                                     rhs=yt_s[:, o, ns_],
                                     start=(o == 0), stop=(o == 1))
                if nt % 2 == 0:
                    nc.scalar.activation(out=kt_s[:, mt, ns_], in_=ps[:],
                                         func=AF.Identity,
                                         bias=bk_s[:, mt:mt + 1])
                else:
                    nc.vector.tensor_scalar_add(kt_s[:, mt, ns_], ps[:],
                                                bk_s[:, mt:mt + 1])
        # V in natural [token, dout] layout, 33-wide head blocks ([Vh | ones])
        v_s = big.tile([P, NKT, H * 33], bf16)
        for tt in range(NKT):
            ts_ = slice(tt * P, tt * P + P)
            ps = proj_pool.tile([P, 512], f32, tag="proj", name="ps")[:, 0:H * 33]
            for o in range(2):
                nc.tensor.matmul(ps[:], lhsT=yt_s[:, o, ts_],
                                 rhs=wv_s[:, o, :], start=(o == 0), stop=(o == 1))
            nc.vector.tensor_copy(out=v_s[:, tt, :], in_=ps[:])
        for h in range(H):
            nc.gpsimd.memset(v_s[:, :, 33 * h + 32], 1.0)

        qt_s = big.tile([P, 4, NTOK], bf16)
        for mt in range(4):
            for nt in range(NQT):
                ns_ = slice(nt * 512, nt * 512 + 512)
                ps = proj_pool.tile([P, 512], f32, tag="proj", name="ps")
                for o in range(2):
                    nc.tensor.matmul(ps[:], lhsT=wq_s[:, o, mt * P:mt * P + P],
                                     rhs=xln_s[:, o, ns_],
                                     start=(o == 0), stop=(o == 1))
                nc.scalar.activation(out=qt_s[:, mt, ns_], in_=ps[:],
                                     func=AF.Identity, bias=bq_s[:, mt:mt + 1])

        # ---- phase B: attention -------------------------------------------
        ot_s = big.tile([P, 4, NTOK], bf16)
        # zero the unwritten strips once (rows 32:64 and 96:128 of each o)
        nc.gpsimd.memset(ot_s[32:64, :, :], 0.0)
        nc.gpsimd.memset(ot_s[96:128, :, :], 0.0)
        rc_s = scratch.tile([P, 512], bf16, tag="rc")
        for pr in range(4):              # head pair: heads {2pr, 2pr+1}
            for qt in range(NQT):
                qs_ = slice(qt * 512, qt * 512 + 512)
                av = av_pool.tile([P, 512], f32, tag="av", name="av")
                for kt in range(NKT):
                    ks_ = slice(kt * P, kt * P + P)
                    sp = scores_pool.tile([P, 1024], f32, tag="scores",
                                          name="sp")
                    for jj in range(2):
                        st = 64 * jj
                        nc.tensor.matmul(
                            sp[:, jj * 512:jj * 512 + 512],
                            lhsT=kt_s[st:st + 32, pr, ks_],
                            rhs=qt_s[st:st + 32, pr, qs_],
                            start=True, stop=True,
                            tile_position=(st, 0))
                    if kt in DVE_KT:
                        ai = apool.tile([P, 1024], i16, tag="a", name="a")
                        nc.vector.tensor_scalar(
                            out=ai[:], in0=sp[:], scalar1=SCH_A, scalar2=SCH_B,
                            op0=ALU.mult, op1=ALU.add)
                        a = ai[:].bitcast(bf16)
                    else:
                        ab = apool.tile([P, 1024], bf16, tag="a", name="a")
                        nc.scalar.activation(out=ab[:], in_=sp[:], func=AF.Exp,
                                             scale=SCALE)
                        a = ab[:]
                    for jj in range(2):
                        h = 2 * pr + jj
                        st = 64 * jj
                        nc.tensor.matmul(
                            av[st:st + 33, :],
                            lhsT=v_s[:, kt, 33 * h:33 * h + 33],
                            rhs=a[:, jj * 512:jj * 512 + 512],
                            start=(kt == 0), stop=(kt == NKT - 1),
                            tile_position=(0, st),
                            skip_group_check=True)
                # normalize by the ones-column sums + per-head residual with Q
                bc = bc_pool.tile([P, 512], f32, tag="bc", name="bc")
                with nc.allow_low_precision(reason="softmax denom recip bf16"):
                    for jj in range(2):
                        st = 64 * jj
                        nc.vector.reciprocal(out=rc_s[st + 32:st + 33, :],
                                             in_=av[st + 32:st + 33, :])
                for jj in range(2):
                    st = 64 * jj
                    nc.tensor.matmul(bc[st:st + 32, :],
                                     lhsT=ones_s[st + 32:st + 33, 0:32],
                                     rhs=rc_s[st + 32:st + 33, :],
                                     start=True, stop=True,
                                     tile_position=(st + 32, st))
                avs = scratch.tile([P, 512], f32, tag="avs", name="avs")
                nrm = scratch.tile([P, 512], bf16, tag="nrm", name="nrm")
                for jj in range(2):
                    st = 64 * jj
                    nc.vector.tensor_copy(out=avs[st:st + 32, :],
                                          in_=av[st:st + 32, :])
                    nc.vector.tensor_tensor(out=nrm[st:st + 32, :],
                                            in0=avs[st:st + 32, :],
                                            in1=bc[st:st + 32, :],
                                            op=ALU.mult)
                    nc.gpsimd.tensor_tensor(out=ot_s[st:st + 32, pr, qs_],
                                            in0=nrm[st:st + 32, :],
                                            in1=qt_s[st:st + 32, pr, qs_],
                                            op=ALU.add)

        # ---- phase C: LN1 + FFN + final residual ---------------------------
        h_s = big.tile([P, 4, NTOK], bf16)
        layernorm(ot_s, oln_s, 4, h_s)
        for mt in range(DFF // P):
            ms = slice(mt * P, mt * P + P)
            for nt in range(NQT):
                ns_ = slice(nt * 512, nt * 512 + 512)
                ps = proj_pool.tile([P, 512], f32, tag="proj", name="ps")
                for o in range(4):
                    nc.tensor.matmul(ps[:], lhsT=w1_s[:, o, ms],
                                     rhs=oln_s[:, o, ns_],
                                     start=(o == 0), stop=(o == 3))
                nc.scalar.activation(out=h_s[:, mt, ns_], in_=ps[:],
                                     func=AF.Gelu, bias=b1_s[:, mt:mt + 1])

        outt_s = big.tile([P, 4, NTOK], f32)
        for mt in range(4):
            ms = slice(mt * P, mt * P + P)
            for nt in range(NQT):
                ns_ = slice(nt * 512, nt * 512 + 512)
                ps = proj_pool.tile([P, 512], f32, tag="proj", name="ps")
                for o in range(4):
                    nc.tensor.matmul(ps[:], lhsT=w2_s[:, o, ms],
                                     rhs=h_s[:, o, ns_],
                                     start=(o == 0), stop=False)
                nc.tensor.matmul(ps[:], lhsT=w2_s[0:1, 4, ms],
                                 rhs=ones_s[0:1, 0:512], start=False, stop=True)
                nc.vector.tensor_tensor(out=outt_s[:, mt, ns_], in0=ps[:],
                                        in1=ot_s[:, mt, ns_], op=ALU.add)
        for h in range(H):
            nc.sync.dma_start(
                out_d[32 * h:32 * h + 32, :],
                outt_s[64 * (h % 2):64 * (h % 2) + 32, h // 2, :])

    nc.compile()
    return nc


def get_nc():
    if "nc" not in _NC_CACHE:
        _NC_CACHE["nc"] = _build_nc()
    return _NC_CACHE["nc"]


def _host_prep(inputs):
    import ml_dtypes

    bf = ml_dtypes.bfloat16
    f = lambda k: np.asarray(inputs[k], np.float32)
    x, y = f("x"), f("y")
    Wq, bq, Wk, bk, Wv, bv = f("Wq"), f("bq"), f("Wk"), f("bk"), f("Wv"), f("bv")
    W1, b1, W2, b2 = f("W1"), f("b1"), f("W2"), f("b2")
    ln0_g, ln0_b, ln1_g, ln1_b = f("ln0_g"), f("ln0_b"), f("ln1_g"), f("ln1_b")
    # fold LN affines into the following linears; fold bv into bq (sum(A)=1)
    Wq_eff = Wq * ln0_g[None, :]
    bq_eff = bq + Wq @ ln0_b + bv
    W1_eff = W1 * ln1_g[None, :]
    b1_eff = b1 + W1 @ ln1_b

    # permutation: original feature d=32h+i -> slot(h,i) in the 512 space
    slots = np.zeros(D, np.int64)
    for h in range(H):
        for i in range(DH):
            slots[DH * h + i] = _slot(h, i)

    wq_h = np.zeros((D, DSLOT), np.float32)
    wq_h[:, slots] = Wq_eff.T            # [din, dout-slot]
    bq_h = np.zeros(DSLOT, np.float32)
    bq_h[slots] = bq_eff
    wk_h = np.zeros((D, DSLOT), np.float32)
    wk_h[:, slots] = Wk.T
    bk_h = np.zeros(DSLOT, np.float32)
    bk_h[slots] = bk
    wv_h = np.zeros((D, H * 33), np.float32)
    for h in range(H):
        wv_h[:, 33 * h:33 * h + 32] = Wv.T[:, DH * h:DH * h + DH]
    w1_h = np.zeros((DSLOT, DFF), np.float32)
    w1_h[slots, :] = W1_eff.T            # [din-slot, dff]
    w2_h = np.zeros((DFF + 1, DSLOT), np.float32)
    w2_h[0:DFF, slots] = W2.T
    w2_h[DFF, slots] = b2

    wq_h = wq_h.astype(bf)
    wk_h = wk_h.astype(bf)
    wv_h = wv_h.astype(bf)
    w1_h = w1_h.astype(bf)
    w2_h = w2_h.astype(bf)

    in_maps = []
    for core in range(8):
        b, half = core // 2, core % 2
        in_maps.append({
            "xt": np.ascontiguousarray(
                x[b, half * NTOK:(half + 1) * NTOK, :].T).astype(bf),
            "yt": np.ascontiguousarray(y[b].T).astype(bf),
            "wq": wq_h, "bq": bq_h, "wk": wk_h, "bk": bk_h, "wv": wv_h,
            "w1": w1_h, "b1": np.ascontiguousarray(b1_eff), "w2": w2_h,
        })
    return in_maps


def kernel_with_results(inputs, **run_kwargs):
    from concourse.bass_utils import run_bass_kernel_spmd
    nc = get_nc()
    in_maps = _host_prep(inputs)
    res = run_bass_kernel_spmd(nc, in_maps, core_ids=list(range(8)), **run_kwargs)
    out = np.empty((B, N, D), np.float32)
    for core in range(8):
        b, half = core // 2, core % 2
        out[b, half * NTOK:(half + 1) * NTOK, :] = res.results[core]["out_t"].T
    return out, res


def kernel(**inputs):
    out, _ = kernel_with_results(inputs)
    return out


# revision 17
# speedup vs baseline: 1.9852x; 1.0201x over previous
"""Trainium2 Bass kernel for a multi-head self-attention block.

Reference computation (B=4, N=2048, D=256, H=8, dh=32, DFF=512):
    x_ln = LN0(x); Q = x_ln@Wq.T+bq; K = y@Wk.T+bk; V = y@Wv.T+bv
    per head: A = softmax(Qh Kh^T / 16); O = concat_h(Qh + A Vh)
    out = O + (gelu(LN1(O)@W1.T+b1) @ W2.T + b2)

Sharding: 8 cores = 4 batches x 2 halves of the query sequence. Each core
gets its x half-shard and the full y for its batch; no collectives.

Layout: feature-on-partition ("transposed") everywhere. The 256 feature
dims of Q/O are spread over a 512-slot space [128 partitions, 4 ktiles]:
head h lives at partition strip 64*(h%2)..+32, ktile o=h//2 (the other
strips are zero). This puts every head's attention output exactly where
the PE col-packed AV matmul (M=33, tile_position col in {0,64}) can
write it, with the softmax denominator coming for free from a ones
column appended to V (row 32/96 of the AV accumulator). LN folds, head
permutation, and the V-bias fold (bv moves into bq since sum(A)=1) are
all host-side weight prep. No max-subtraction in softmax (|s/16|<~1.5).

All matmul operands are bf16 (1 PE cycle/row vs 4 for fp32); PSUM
accumulation stays fp32. The softmax exp is split across engines: the
Act engine does exact Exp on most key-tiles, the DVE computes the rest
with a Schraudolph-style exp (one tensor_scalar building bf16 bit
patterns in int16; the +-3% equi-ripple error is common-mode across
the softmax and mostly cancels). GPSIMD takes the all-SBUF elementwise
work (LN normalize, residual adds) since it cannot touch PSUM.

Scheduling: engines have in-order queues, so an instruction waiting on
a long dependency chain blocks everything behind it on that queue.
Three restructures keep the PE fed: (1) K/V projection matmuls are
issued between the LN0 sums and the LN0 broadcast matmuls so the PE
works while the LN scalar chain runs; (2) each attention iteration's
epilogue (denominator reciprocal -> broadcast -> normalize) is deferred
into the next iteration's key loop, so its PE broadcast never waits at
the queue head; (3) the query loop is qt-major and LN1/FFN/output-DMA
are chunked per 512 tokens and interleaved into the tail attention
iterations, hiding most of phase C. Softmax reciprocals use the ~51-ULP
approx DVE op (5x faster than the bit-exact iterative divide).
"""

import contextlib

import numpy as np

B, N, D = 4, 2048, 256
H, DH, DFF = 8, 32, 512
P = 128
NTOK = N // 2            # query tokens per core
NQT = NTOK // 512        # q tiles of 512
NKT = N // P             # key tiles of 128
SCALE = 1.0 / 16.0
EPS = 1e-5
DSLOT = 512              # padded feature-slot space for Q/K/O

# Schraudolph exp constants for bf16 bit patterns in int16:
#   bits = round(s * SCALE*128*log2(e) + (127*128 - c8))
LOG2E = 1.4426950408889634
SCH_A = SCALE * 128.0 * LOG2E
SCH_B = 127.0 * 128.0 - 366392.5 / 65536.0
# key tiles handled by the DVE (Schraudolph) instead of Act (exact exp)
DVE_KT = frozenset((2, 5, 8, 11, 13, 15))

_NC_CACHE = {}


def _slot(h, i):
    return (h // 2) * P + 64 * (h % 2) + i


def _build_nc():
    import concourse.mybir as mybir
    import concourse.tile as tile
    from concourse import bacc

    f32 = mybir.dt.float32
    bf16 = mybir.dt.bfloat16
    i16 = mybir.dt.int16
    AF = mybir.ActivationFunctionType
    ALU = mybir.AluOpType

    nc = bacc.Bacc("TRN2", target_bir_lowering=False, debug=False)

    xt_d = nc.dram_tensor("xt", [D, NTOK], bf16, kind="ExternalInput")
    yt_d = nc.dram_tensor("yt", [D, N], bf16, kind="ExternalInput")
    wq_d = nc.dram_tensor("wq", [D, DSLOT], bf16, kind="ExternalInput")
    bq_d = nc.dram_tensor("bq", [DSLOT], f32, kind="ExternalInput")
    wk_d = nc.dram_tensor("wk", [D, DSLOT], bf16, kind="ExternalInput")
    bk_d = nc.dram_tensor("bk", [DSLOT], f32, kind="ExternalInput")
    wv_d = nc.dram_tensor("wv", [D, H * 33], bf16, kind="ExternalInput")
    w1_d = nc.dram_tensor("w1", [DSLOT, DFF], bf16, kind="ExternalInput")
    b1_d = nc.dram_tensor("b1", [DFF], f32, kind="ExternalInput")
    w2_d = nc.dram_tensor("w2", [DFF + 1, DSLOT], bf16, kind="ExternalInput")
    out_d = nc.dram_tensor("out_t", [D, NTOK], f32, kind="ExternalOutput")

    with tile.TileContext(nc) as tc, contextlib.ExitStack() as ctx:
        const = ctx.enter_context(tc.tile_pool(name="const", bufs=1))
        big = ctx.enter_context(tc.tile_pool(name="big", bufs=1))
        scratch = ctx.enter_context(tc.tile_pool(name="scratch", bufs=1))
        apool = ctx.enter_context(tc.tile_pool(name="apool", bufs=4))
        # PSUM: scores 2x[128,1024]=4 banks, av 2, bc 1, proj 1.
        scores_pool = ctx.enter_context(
            tc.tile_pool(name="scoresp", bufs=2, space="PSUM"))
        av_pool = ctx.enter_context(tc.tile_pool(name="avp", bufs=2, space="PSUM"))
        bc_pool = ctx.enter_context(tc.tile_pool(name="bcp", bufs=1, space="PSUM"))
        proj_pool = ctx.enter_context(tc.tile_pool(name="projp", bufs=1, space="PSUM"))

        # ---- constants / inputs -------------------------------------------
        ones_s = const.tile([P, 512], bf16)
        nc.vector.memset(ones_s[:], 1.0)
        eps_s = const.tile([1, 1], f32)
        nc.vector.memset(eps_s[:], EPS)

        xt_s = big.tile([P, 2, NTOK], bf16)
        nc.sync.dma_start(xt_s[:], xt_d.rearrange("(o p) t -> p o t", p=P))
        yt_s = big.tile([P, 2, N], bf16)
        nc.sync.dma_start(yt_s[:], yt_d.rearrange("(o p) t -> p o t", p=P))

        wk_s = const.tile([P, 2, DSLOT], bf16)
        nc.sync.dma_start(wk_s[:], wk_d.rearrange("(o p) m -> p o m", p=P))
        wv_s = const.tile([P, 2, H * 33], bf16)
        nc.sync.dma_start(wv_s[:], wv_d.rearrange("(o p) m -> p o m", p=P))
        wq_s = const.tile([P, 2, DSLOT], bf16)
        nc.sync.dma_start(wq_s[:], wq_d.rearrange("(o p) m -> p o m", p=P))
        w1_s = const.tile([P, 4, DFF], bf16)
        nc.sync.dma_start(w1_s[:], w1_d.rearrange("(o p) m -> p o m", p=P))
        w2_s = const.tile([P, 5, DSLOT], bf16)
        nc.sync.dma_start(w2_s[:, 0:4, :],
                          w2_d[0:DFF, :].rearrange("(o p) m -> p o m", p=P))
        nc.sync.dma_start(w2_s[0:1, 4, :], w2_d[DFF:, :])
        bq_s = const.tile([P, 4], f32)
        nc.sync.dma_start(bq_s[:], bq_d.rearrange("(m p) -> p m", p=P))
        bk_s = const.tile([P, 4], f32)
        nc.sync.dma_start(bk_s[:], bk_d.rearrange("(m p) -> p m", p=P))
        b1_s = const.tile([P, 4], f32)
        nc.sync.dma_start(b1_s[:], b1_d.rearrange("(m p) -> p m", p=P))

        # ---- shared LN scratch --------------------------------------------
        mean = scratch.tile([1, NTOK], f32, tag="mean")
        mean_b = scratch.tile([1, NTOK], bf16, tag="mean_b")
        rstd_b = scratch.tile([1, NTOK], bf16, tag="rstd_b")
        lt = scratch.tile([1, NTOK], f32, tag="lntmp")
        m2 = scratch.tile([1, NTOK], f32, tag="m2")
        rsf = scratch.tile([1, NTOK], f32, tag="rsf")
        mb_sb = scratch.tile([P, NTOK], bf16, tag="mb_sb")
        rb_sb = scratch.tile([P, NTOK], bf16, tag="rb_sb")

        def ln_sums(src, sq, no, hf):
            """Square already computed into sq; accumulate chunk sums and
            produce mean / E[x^2] for token chunk hf. Both sums live in ONE
            proj_pool tile (sx at row 0, sq at row 32 via col tile_position)
            so this never disturbs the av/bc pool rotation that in-flight
            attention epilogues depend on."""
            cs = slice(hf * 512, hf * 512 + 512)
            ps = proj_pool.tile([P, 512], f32, tag="proj", name="lnsum")
            for o in range(no):
                nc.tensor.matmul(ps[0:1, :], lhsT=ones_s[:, 0:1],
                                 rhs=src[:, o, cs],
                                 start=(o == 0), stop=(o == no - 1),
                                 tile_position=(0, 0), skip_group_check=True)
                nc.tensor.matmul(ps[32:33, :], lhsT=ones_s[:, 0:1],
                                 rhs=sq[:, o, cs],
                                 start=(o == 0), stop=(o == no - 1),
                                 tile_position=(0, 32), skip_group_check=True)
            nc.vector.tensor_scalar_mul(mean[0:1, cs], ps[0:1, :], 1.0 / D)
            nc.vector.tensor_scalar_mul(lt[0:1, cs], ps[32:33, :], 1.0 / D)

        def ln_finish(src, dst, no, hf):
            """rstd for chunk hf, broadcast, normalize src->dst (GPSIMD)."""
            cs = slice(hf * 512, hf * 512 + 512)
            nc.vector.tensor_tensor(out=m2[0:1, cs], in0=mean[0:1, cs],
                                    in1=mean[0:1, cs], op=ALU.mult)
            nc.vector.tensor_tensor(out=lt[0:1, cs], in0=lt[0:1, cs],
                                    in1=m2[0:1, cs], op=ALU.subtract)
            nc.scalar.activation(out=lt[0:1, cs], in_=lt[0:1, cs], func=AF.Sqrt,
                                 bias=eps_s[:])
            nc.vector.reciprocal(out=rsf[0:1, cs], in_=lt[0:1, cs])
            nc.vector.tensor_copy(out=rstd_b[0:1, cs], in_=rsf[0:1, cs])
            nc.vector.tensor_copy(out=mean_b[0:1, cs], in_=mean[0:1, cs])
            br = scores_pool.tile([P, 1024], f32, tag="scores", name="br")
            nc.tensor.matmul(br[:, 0:512], lhsT=ones_s[0:1, 0:P],
                             rhs=mean_b[0:1, cs], start=True, stop=True)
            nc.tensor.matmul(br[:, 512:1024], lhsT=ones_s[0:1, 0:P],
                             rhs=rstd_b[0:1, cs], start=True, stop=True)
            nc.scalar.activation(out=mb_sb[:, cs], in_=br[:, 0:512], func=AF.Copy)
            nc.scalar.activation(out=rb_sb[:, cs], in_=br[:, 512:1024],
                                 func=AF.Copy)
            for o in range(no):
                nc.gpsimd.tensor_tensor(out=dst[:, o, cs], in0=src[:, o, cs],
                                        in1=mb_sb[:, cs], op=ALU.subtract)
                nc.gpsimd.tensor_tensor(out=dst[:, o, cs], in0=dst[:, o, cs],
                                        in1=rb_sb[:, cs], op=ALU.mult)

        # ---- phase A: LN0 (sums), K/V proj, LN0 finish, Q proj -------------
        xln_s = big.tile([P, 2, NTOK], bf16)
        oln_s = big.tile([P, 4, NTOK], bf16)
        sq0 = oln_s[:, 0:2, :]                 # borrow as Square scratch
        nc.scalar.activation(out=sq0[:], in_=xt_s[:], func=AF.Square)
        for hf in range(NQT):
            ln_sums(xt_s, sq0, 2, hf)

        # K/V proj keep the PE busy while the LN0 scalar chain runs.
        kt_s = big.tile([P, 4, N], bf16)
        for mt in range(4):
            for nt in range(N // 512):
                ns_ = slice(nt * 512, nt * 512 + 512)
                ps = proj_pool.tile([P, 512], f32, tag="proj", name="ps")
                for o in range(2):
                    nc.tensor.matmul(ps[:], lhsT=wk_s[:, o, mt * P:mt * P + P],
                                     rhs=yt_s[:, o, ns_],
                                     start=(o == 0), stop=(o == 1))
                if nt % 2 == 0:
                    nc.scalar.activation(out=kt_s[:, mt, ns_], in_=ps[:],
                                         func=AF.Identity,
                                         bias=bk_s[:, mt:mt + 1])
                else:
                    nc.vector.tensor_scalar_add(kt_s[:, mt, ns_], ps[:],
                                                bk_s[:, mt:mt + 1])
        # V in natural [token, dout] layout, 33-wide head blocks ([Vh | ones])
        v_s = big.tile([P, NKT, H * 33], bf16)
        for tt in range(NKT):
            ts_ = slice(tt * P, tt * P + P)
            ps = proj_pool.tile([P, 512], f32, tag="proj", name="ps")[:, 0:H * 33]
            for o in range(2):
                nc.tensor.matmul(ps[:], lhsT=yt_s[:, o, ts_],
                                 rhs=wv_s[:, o, :], start=(o == 0), stop=(o == 1))
            if tt % 2 == 0:
                nc.scalar.activation(out=v_s[:, tt, :], in_=ps[:], func=AF.Copy)
            else:
                nc.vector.tensor_copy(out=v_s[:, tt, :], in_=ps[:])
        for h in range(H):
            nc.gpsimd.memset(v_s[:, :, 33 * h + 32], 1.0)

        for hf in range(NQT):
            ln_finish(xt_s, xln_s, 2, hf)

        qt_s = big.tile([P, 4, NTOK], bf16)
        for mt in range(4):
            for nt in range(NQT):
                ns_ = slice(nt * 512, nt * 512 + 512)
                ps = proj_pool.tile([P, 512], f32, tag="proj", name="ps")
                for o in range(2):
                    nc.tensor.matmul(ps[:], lhsT=wq_s[:, o, mt * P:mt * P + P],
                                     rhs=xln_s[:, o, ns_],
                                     start=(o == 0), stop=(o == 1))
                nc.scalar.activation(out=qt_s[:, mt, ns_], in_=ps[:],
                                     func=AF.Identity, bias=bq_s[:, mt:mt + 1])

        # ---- phase B (attention) with phase C (LN1+FFN) interleaved --------
        ot_s = big.tile([P, 4, NTOK], bf16)
        nc.gpsimd.memset(ot_s[32:64, :, :], 0.0)
        nc.gpsimd.memset(ot_s[96:128, :, :], 0.0)
        h_s = big.tile([P, 4, NTOK], bf16)
        outt_s = big.tile([P, 4, NTOK], f32)
        rcf_s = scratch.tile([P, 512], f32, tag="rcf")
        rc_s = scratch.tile([P, 512], bf16, tag="rc")

        def make_epilogue(pr, qt, av):
            qs_ = slice(qt * 512, qt * 512 + 512)

            def emit():
                for jj in range(2):
                    st = 64 * jj
                    nc.vector.reciprocal(out=rcf_s[st + 32:st + 33, :],
                                         in_=av[st + 32:st + 33, :])
                    nc.vector.tensor_copy(out=rc_s[st + 32:st + 33, :],
                                          in_=rcf_s[st + 32:st + 33, :])
                bc = bc_pool.tile([P, 512], f32, tag="bc", name="bc")
                for jj in range(2):
                    st = 64 * jj
                    nc.tensor.matmul(bc[st:st + 32, :],
                                     lhsT=ones_s[st + 32:st + 33, 0:32],
                                     rhs=rc_s[st + 32:st + 33, :],
                                     start=True, stop=True,
                                     tile_position=(st + 32, st))
                avs = scratch.tile([P, 512], f32, tag="avs", name="avs")
                nrm = scratch.tile([P, 512], bf16, tag="nrm", name="nrm")
                for jj in range(2):
                    st = 64 * jj
                    nc.vector.tensor_copy(out=avs[st:st + 32, :],
                                          in_=av[st:st + 32, :])
                    nc.vector.tensor_tensor(out=nrm[st:st + 32, :],
                                            in0=avs[st:st + 32, :],
                                            in1=bc[st:st + 32, :],
                                            op=ALU.mult)
                    nc.gpsimd.tensor_tensor(out=ot_s[st:st + 32, pr, qs_],
                                            in0=nrm[st:st + 32, :],
                                            in1=qt_s[st:st + 32, pr, qs_],
                                            op=ALU.add)
            return emit

        def ffn1_chunk(hf):
            cs = slice(hf * 512, hf * 512 + 512)
            for mt in range(DFF // P):
                ms = slice(mt * P, mt * P + P)
                ps = proj_pool.tile([P, 512], f32, tag="proj", name="ps")
                for o in range(4):
                    nc.tensor.matmul(ps[:], lhsT=w1_s[:, o, ms],
                                     rhs=oln_s[:, o, cs],
                                     start=(o == 0), stop=(o == 3))
                nc.scalar.activation(out=h_s[:, mt, cs], in_=ps[:],
                                     func=AF.Gelu, bias=b1_s[:, mt:mt + 1])

        def ffn2_chunk(hf):
            cs = slice(hf * 512, hf * 512 + 512)
            for mt in range(4):
                ms = slice(mt * P, mt * P + P)
                ps = proj_pool.tile([P, 512], f32, tag="proj", name="ps")
                for o in range(4):
                    nc.tensor.matmul(ps[:], lhsT=w2_s[:, o, ms],
                                     rhs=h_s[:, o, cs],
                                     start=(o == 0), stop=False)
                nc.tensor.matmul(ps[:], lhsT=w2_s[0:1, 4, ms],
                                 rhs=ones_s[0:1, 0:512], start=False, stop=True)
                nc.vector.tensor_tensor(out=outt_s[:, mt, cs], in0=ps[:],
                                        in1=ot_s[:, mt, cs], op=ALU.add)
            for h in range(H):
                nc.sync.dma_start(
                    out_d[32 * h:32 * h + 32, cs],
                    outt_s[64 * (h % 2):64 * (h % 2) + 32, h // 2, cs])

        def ln1_square_sums(hf):
            nc.scalar.activation(out=h_s[:, :, hf * 512:hf * 512 + 512],
                                 in_=ot_s[:, :, hf * 512:hf * 512 + 512],
                                 func=AF.Square)
            ln_sums(ot_s, h_s, 4, hf)

        pending = None       # previous iteration's epilogue
        deferred = []        # chunked LN1/FFN stages

        for idx, (qt, pr) in enumerate(
                [(q, p) for q in range(NQT) for p in range(4)]):
            qs_ = slice(qt * 512, qt * 512 + 512)
            av = av_pool.tile([P, 512], f32, tag="av", name="av")
            for kt in range(NKT):
                ks_ = slice(kt * P, kt * P + P)
                sp = scores_pool.tile([P, 1024], f32, tag="scores", name="sp")
                for jj in range(2):
                    st = 64 * jj
                    nc.tensor.matmul(
                        sp[:, jj * 512:jj * 512 + 512],
                        lhsT=kt_s[st:st + 32, pr, ks_],
                        rhs=qt_s[st:st + 32, pr, qs_],
                        start=True, stop=True,
                        tile_position=(st, 0))
                if kt in DVE_KT:
                    ai = apool.tile([P, 1024], i16, tag="a", name="a")
                    nc.vector.tensor_scalar(
                        out=ai[:], in0=sp[:], scalar1=SCH_A, scalar2=SCH_B,
                        op0=ALU.mult, op1=ALU.add)
                    a = ai[:].bitcast(bf16)
                else:
                    ab = apool.tile([P, 1024], bf16, tag="a", name="a")
                    nc.scalar.activation(out=ab[:], in_=sp[:], func=AF.Exp,
                                         scale=SCALE)
                    a = ab[:]
                for jj in range(2):
                    h = 2 * pr + jj
                    st = 64 * jj
                    nc.tensor.matmul(
                        av[st:st + 33, :],
                        lhsT=v_s[:, kt, 33 * h:33 * h + 33],
                        rhs=a[:, jj * 512:jj * 512 + 512],
                        start=(kt == 0), stop=(kt == NKT - 1),
                        tile_position=(0, st),
                        skip_group_check=True)
                if kt == 3 and pending is not None:
                    pending()
                    pending = None
                elif kt in (8, 12) and deferred:
                    deferred.pop(0)()
            pending = make_epilogue(pr, qt, av)
            if idx == 3:
                # chunk 0 (qt=0 tokens) post-processing, interleaved into the
                # qt=1 iterations; stages become ready as epilogues land.
                deferred.extend([
                    lambda: ln1_square_sums(0),
                    lambda: ln_finish(ot_s, oln_s, 4, 0),
                    lambda: ffn1_chunk(0),
                    lambda: ffn2_chunk(0),
                ])
        pending()
        ln1_square_sums(1)
        ln_finish(ot_s, oln_s, 4, 1)
        ffn1_chunk(1)
        ffn2_chunk(1)

    nc.compile()
    return nc


def get_nc():
    if "nc" not in _NC_CACHE:
        _NC_CACHE["nc"] = _build_nc()
    return _NC_CACHE["nc"]


def _host_prep(inputs):
    import ml_dtypes

    bf = ml_dtypes.bfloat16
    f = lambda k: np.asarray(inputs[k], np.float32)
    x, y = f("x"), f("y")
    Wq, bq, Wk, bk, Wv, bv = f("Wq"), f("bq"), f("Wk"), f("bk"), f("Wv"), f("bv")
    W1, b1, W2, b2 = f("W1"), f("b1"), f("W2"), f("b2")
    ln0_g, ln0_b, ln1_g, ln1_b = f("ln0_g"), f("ln0_b"), f("ln1_g"), f("ln1_b")
    # fold LN affines into the following linears; fold bv into bq (sum(A)=1)
    Wq_eff = Wq * ln0_g[None, :]
    bq_eff = bq + Wq @ ln0_b + bv
    W1_eff = W1 * ln1_g[None, :]
    b1_eff = b1 + W1 @ ln1_b

    # permutation: original feature d=32h+i -> slot(h,i) in the 512 space
    slots = np.zeros(D, np.int64)
    for h in range(H):
        for i in range(DH):
            slots[DH * h + i] = _slot(h, i)

    wq_h = np.zeros((D, DSLOT), np.float32)
    wq_h[:, slots] = Wq_eff.T            # [din, dout-slot]
    bq_h = np.zeros(DSLOT, np.float32)
    bq_h[slots] = bq_eff
    wk_h = np.zeros((D, DSLOT), np.float32)
    wk_h[:, slots] = Wk.T
    bk_h = np.zeros(DSLOT, np.float32)
    bk_h[slots] = bk
    wv_h = np.zeros((D, H * 33), np.float32)
    for h in range(H):
        wv_h[:, 33 * h:33 * h + 32] = Wv.T[:, DH * h:DH * h + DH]
    w1_h = np.zeros((DSLOT, DFF), np.float32)
    w1_h[slots, :] = W1_eff.T            # [din-slot, dff]
    w2_h = np.zeros((DFF + 1, DSLOT), np.float32)
    w2_h[0:DFF, slots] = W2.T
    w2_h[DFF, slots] = b2

    wq_h = wq_h.astype(bf)
    wk_h = wk_h.astype(bf)
    wv_h = wv_h.astype(bf)
    w1_h = w1_h.astype(bf)
    w2_h = w2_h.astype(bf)

    in_maps = []
    for core in range(8):
        b, half = core // 2, core % 2
        in_maps.append({
            "xt": np.ascontiguousarray(
                x[b, half * NTOK:(half + 1) * NTOK, :].T).astype(bf),
            "yt": np.ascontiguousarray(y[b].T).astype(bf),
            "wq": wq_h, "bq": bq_h, "wk": wk_h, "bk": bk_h, "wv": wv_h,
            "w1": w1_h, "b1": np.ascontiguousarray(b1_eff), "w2": w2_h,
        })
    return in_maps


def kernel_with_results(inputs, **run_kwargs):
    from concourse.bass_utils import run_bass_kernel_spmd
    nc = get_nc()
    in_maps = _host_prep(inputs)
    res = run_bass_kernel_spmd(nc, in_maps, core_ids=list(range(8)), **run_kwargs)
    out = np.empty((B, N, D), np.float32)
    for core in range(8):
        b, half = core // 2, core % 2
        out[b, half * NTOK:(half + 1) * NTOK, :] = res.results[core]["out_t"].T
    return out, res


def kernel(**inputs):
    out, _ = kernel_with_results(inputs)
    return out
